# revision 33
# baseline (speedup 1.0000x reference)
"""Trainium2 Bass kernel: multi-head attention block (DiyTransformer).

Full-input contract: kernel(**inputs) takes the unsharded inputs and returns
the full [2, 2048, 1024] output. Internally shards 16 heads across 8
NeuronCores (2 heads = one 128-wide feature slice per core).

Math (reference):
  q = x @ wq.T + bq ; k = x @ wk.T + bk ; v = x @ wv.T + bv   (per-head split)
  out_h = softmax(q_h k_h^T / 8) v_h ;  y = concat(out_h) @ wo.T + bo

Performance structure (defaults):
  - q/k projections in fp8e4 DoubleRow (2 contraction tiles per matmul,
    weights prescaled by 2^13 / 2^10 into fp8 range); x is fed both as
    bf16 (v projection) and fp8 (q/k). The prescale is NOT descaled at the
    psum evacuation (a plain ACT copy / bias-add) - it is folded into the
    exp argument (ACT activation scale=2^-23, Schraudolph EXP_A).
  - pipe_attn: software-pipelined attention. Per period, the PE stream is
    [16 single-chunk score groups of block k | 32 PV matmuls of block k-1
    interleaved into the gaps | 8 out-proj matmuls of block k-2], so the
    PE never waits on exp: score group g is psum-gated on exp group g-2 of
    the same block (pools s0/s1 alternate, 2 banks each), the PV consumes
    exp tiles completed last period (exp_pool bufs=3), and the out-proj
    runs two periods after its PV so the den->recip->bc2->oT chain is
    fully hidden. ps_pv has 4 bufs (pvs x2 + po rotation).
  - exp: split ACT (Exp activation, scale-folded) / DVE (Schraudolph
    bit-trick exp: round(x*128/ln2 + B) -> int16 == bf16 bits, 1.8% rms);
    pipe_dve of the 16 chunk-units per block go to the DVE - on HW the ACT
    is the attention-phase bottleneck so a high split (6-10) wins.
  - score matmuls of the 2 heads are PE row-tiled (partition bases 0/64)
    into disjoint quadrants -> concurrent (measured ~31us win).
  - output in fp16 (halves the 16MB/core output DMA; partials are ~1e-1
    so fp16 rounding is negligible); host sums the 8 cores' partials in
    f32 and adds bo_eff.
  - inputs one packed DMA per weight + 8 seq-range xT DMAs so projections
    start as soon as the first slice lands; output DMAs merged [128,1024]
    and issued from the gpsimd queue. x8_onchip converts bf16->fp8 on
    DVE+ACT instead of DMAing a second 4MB copy of x.

Simplifications used here:
  - k bias: adds a per-query constant to every logit in a softmax row ->
    cancels exactly; dropped.
  - v bias: softmax rows sum to 1, so attn @ (v + bv) = attn @ v + bv.
    The bv term is folded into a host-side constant bo_eff = bo + bv @ wo.T.
  - 1/8 scale folded into wq and bq on the host.
  - scores are computed transposed (scoresT[k_pos, q] = k @ qT), so softmax's
    sum runs along the PSUM partition dim. A ones-column appended to v makes
    the PV matmul emit the denominator for free (row 64 of the PV psum), and
    no PE transposes are needed anywhere in the pipeline. (Engines can only
    address 32-aligned partition bases, and the custom-DVE reciprocal drops
    nonzero input bases on HW, so the denominator is staged to partition 0
    before the reciprocal — both heads merged into one staging tile.)

v2 structure changes vs the first working version:
  - host pre-packs xT/wq/wk/wv into SBUF-layout order so each input is ONE
    contiguous DMA (xT: 8 seq-range DMAs so Q/K projection of seq block jj
    can start as soon as its slice lands, instead of after the full 8MB).
  - output DMAs are [128,1024] (2 psum banks staged into one tile) and are
    issued from the gpsimd (Pool) queue, off the busy SP queue.
  - denominator: ones column first -> no den staging copies; reciprocal runs
    directly on psum row 0; one broadcast DMA per block for both heads.
  - v-projection evacuation: one 3D-AP copy per 128-seq chunk (both heads).
  - a dummy 1-wide exp early in the program pulls the ACT table load into
    the input-DMA window.
"""

import sys

sys.path.insert(0, "/opt/trn_rl_repo")

import zlib

import numpy as np
import ml_dtypes

# The axon terminal caches compiled executables by module name + I/O
# signature only (the BIR payload in backend_config is not in the key), so a
# changed kernel with unchanged tensor shapes silently reuses the stale
# executable. Bust it by adding a dummy input whose shape encodes a hash of
# this file's source.
with open(__file__, "rb") as _f:
    _VTAG = (zlib.crc32(_f.read()) % 4093) + 3

D = 1024          # embed dim
NH = 16           # total heads
DH = 64           # head dim
NB = 2            # batch
S = 2048          # seq len
M = NB * S        # 4096 flattened rows
N_CORES = 8
HPC = 2           # heads per core
FS = HPC * DH     # 128 per-core feature slice
DCH = D // 128    # 8 contraction chunks
SCALE = 1.0 / np.sqrt(DH)

BF16 = ml_dtypes.bfloat16

_compiled = None  # (nc, module) cache


QK_SHIFT = 8192.0    # 2**13, q-weight prescale for fp8 (q includes 1/8 scale)
K_SHIFT = 1024.0     # 2**10, k-weight prescale for fp8


def _build(repeat=1, dve_exp=2, interleave=False, defer_po=False,
           recip_direct=False, skip_out=False, serial_scores=False,
           fp8_qk=True, x8_onchip=True, vtag_extra=0,
           f16_out=True, act_evac=True, oc_split=True,
           pipe_attn=True, pipe_dve=6, pipe_ilv=True):
    import concourse.bass as bass
    import concourse.tile as tile
    from concourse import bacc, mybir

    f32 = mybir.dt.float32
    bf16 = mybir.dt.bfloat16
    f16 = mybir.dt.float16
    f8e4 = mybir.dt.float8e4
    out_dt = f16 if f16_out else f32
    # act_evac: q/k psum evacuation on ACT as pure copy/bias-add; the fp8
    # weight prescale (2^13 * 2^10) is NOT descaled at evacuation — it is
    # folded into the exp argument scale instead (ACT activation scale=,
    # Schraudolph EXP_A). Requires fp8_qk.
    evac_fold = act_evac and fp8_qk
    exp_scale = 1.0 / (QK_SHIFT * K_SHIFT) if evac_fold else 1.0

    nc = bacc.Bacc("TRN2", target_bir_lowering=False, debug=False,
                   num_devices=N_CORES)

    # all in SBUF-ready layouts (host pre-packed)
    xT_d = nc.dram_tensor("xT", [128, DCH * M], bf16, kind="ExternalInput").ap()
    qk_dt = f8e4 if fp8_qk else bf16
    if fp8_qk and not x8_onchip:
        x8_d = nc.dram_tensor("x8", [128, DCH * M], f8e4,
                              kind="ExternalInput").ap()
    wq_d = nc.dram_tensor("wqT", [128, D], qk_dt, kind="ExternalInput").ap()
    wk_d = nc.dram_tensor("wkT", [128, D], qk_dt, kind="ExternalInput").ap()
    wv_d = nc.dram_tensor("wvT", [128, D], bf16, kind="ExternalInput").ap()
    wo_d = nc.dram_tensor("woT", [FS, D], bf16, kind="ExternalInput").ap()
    bq_d = nc.dram_tensor("bq", [FS, 1], f32, kind="ExternalInput").ap()
    nc.dram_tensor("vtag", [1, _VTAG + vtag_extra + (repeat - 1) * 4096], f32,
                   kind="ExternalInput")
    out_d = nc.dram_tensor("out", [M, D], out_dt, kind="ExternalOutput").ap()

    Exp = mybir.ActivationFunctionType.Exp

    def ap3(t_ap, extra_off, dims):
        return bass.AP(t_ap.tensor, t_ap.offset + extra_off,
                       [list(t_ap.ap[0])] + [list(d) for d in dims])

    with tile.TileContext(nc) as tc:
        with (
            tc.tile_pool(name="persist", bufs=1) as persist,
            tc.tile_pool(name="stage", bufs=3 if fp8_qk else 4) as stage,
            tc.tile_pool(name="exp", bufs=3 if pipe_attn else 2) as exp_pool,
            tc.tile_pool(name="oT", bufs=2) as oT_pool,
            tc.tile_pool(name="smalls", bufs=1 if fp8_qk else 2) as smalls,
            tc.tile_pool(name="ps_s0", bufs=1, space="PSUM") as ps_s0,
            tc.tile_pool(name="ps_s1", bufs=1, space="PSUM") as ps_s1,
            tc.tile_pool(name="ps_pv", bufs=4 if pipe_attn else 2,
                         space="PSUM") as ps_pv,
        ):
            for _rep in range(repeat):
                # ---- load inputs to SBUF (weights first; xT by seq range) ----
                wq = persist.tile([128, D], qk_dt, tag="wq")
                wk = persist.tile([128, D], qk_dt, tag="wk")
                wv = persist.tile([128, D], bf16, tag="wv")
                wo = persist.tile([128, D], bf16, tag="wo")
                bq = persist.tile([FS, 1], f32, tag="bq")
                xT = persist.tile([128, DCH * M], bf16, tag="xT")  # [d-chunk|seq]
                xT_all = xT[:, :]
                if fp8_qk:
                    x8 = persist.tile([128, DCH * M], f8e4, tag="x8")
                    x8_all = x8[:, :]

                Copy = mybir.ActivationFunctionType.Copy

                def xT_dma(jj):
                    off = jj * 512
                    if fp8_qk and not x8_onchip:
                        nc.sync.dma_start(
                            ap3(x8_all, off, [[M, DCH], [1, 512]]),
                            ap3(x8_d, off, [[M, DCH], [1, 512]]))
                    nc.sync.dma_start(
                        ap3(xT_all, off, [[M, DCH], [1, 512]]),
                        ap3(xT_d, off, [[M, DCH], [1, 512]]))
                    if fp8_qk and x8_onchip:
                        # The projection phase is input-DMA-bound: convert
                        # this seq-range of xT to fp8 on-chip instead of
                        # DMAing a second 4MB copy of x from HBM, split
                        # DVE (5 of 8 d-chunks) / ACT (3 of 8) so neither
                        # engine paces the phase.
                        nc.vector.tensor_copy(
                            ap3(x8_all, off, [[M, 5], [1, 512]]),
                            ap3(xT_all, off, [[M, 5], [1, 512]]))
                        nc.scalar.activation(
                            ap3(x8_all, 5 * M + off, [[M, 3], [1, 512]]),
                            ap3(xT_all, 5 * M + off, [[M, 3], [1, 512]]),
                            Copy)

                nc.sync.dma_start(bq[:, :], bq_d[:, :])
                nc.sync.dma_start(wq[:, :], wq_d[:, :])
                xT_dma(0)
                nc.sync.dma_start(wk[:, :], wk_d[:, :])
                nc.sync.dma_start(wv[:, :], wv_d[:, :])
                xT_dma(1)
                nc.sync.dma_start(wo[:, :], wo_d[:, :])

                # warm the exp table while DMAs run
                warm = smalls.tile([1, 1], f32, tag="warm")
                nc.vector.memset(warm[:, :], 0.0)
                nc.scalar.activation(warm[:, :], warm[:, :], Exp)

                for jj in range(2, 8):
                    xT_dma(jj)

                # ---- projections ----
                qT = persist.tile([128, M], bf16, tag="qT")   # [feat, seq]
                kT = persist.tile([128, M], bf16, tag="kT")
                # v natural layout + ones column: slot(h,c) = h*32+c, 65 wide
                vv = persist.tile([128, HPC * 32 * 65], bf16, tag="v")
                if pipe_attn:
                    # only the 64 per-slot ones-columns need initializing
                    # (the 64-wide v slices are fully written by the
                    # projection evacuation) - a full-tile memset is ~0.5M
                    # elements of DVE time
                    nc.vector.memset(
                        ap3(vv[:, :], 64, [[65, HPC * 32], [1, 1]]), 1.0)
                else:
                    nc.vector.memset(vv[:, :], 1.0)

                DR = mybir.MatmulPerfMode.DoubleRow

                def qk_matmuls(ps_out, w, jj):
                    if fp8_qk:
                        # DoubleRow: two 128-row k-tiles per matmul; both the
                        # packed weight layout [p, d*128+f] and the x8 layout
                        # [p, d*M+s] already place tile pairs at the right
                        # free-dim strides.
                        for d2 in range(DCH // 2):
                            nc.tensor.matmul(
                                ps_out,
                                ap3(w[:, :], d2 * 256, [[128, 2], [1, 128]]),
                                ap3(x8_all, 2 * d2 * M + jj * 512,
                                    [[M, 2], [1, 512]]),
                                start=(d2 == 0), stop=(d2 == DCH // 2 - 1),
                                perf_mode=DR)
                    else:
                        for d in range(DCH):
                            nc.tensor.matmul(
                                ps_out, w[:, d * 128:(d + 1) * 128],
                                xT[:, d * M + jj * 512: d * M + (jj + 1) * 512],
                                start=(d == 0), stop=(d == DCH - 1))

                Identity = mybir.ActivationFunctionType.Identity

                def proj_block(jj):
                    qs = slice(jj * 512, (jj + 1) * 512)
                    pq = ps_pv.tile([128, 512], f32, tag="pv")
                    qk_matmuls(pq[:, :], wq, jj)
                    if evac_fold:
                        # ACT evacuation (DVE is the proj-phase bottleneck
                        # otherwise): qT keeps the 2^13 weight prescale; the
                        # host passes bq prescaled to match, and exp descales.
                        nc.scalar.activation(qT[:, qs], pq[:, :], Identity,
                                             bias=bq[:, 0:1])
                    elif fp8_qk:
                        nc.vector.tensor_scalar(
                            qT[:, qs], pq[:, :], 1.0 / QK_SHIFT, bq[:, 0:1],
                            mybir.AluOpType.mult, mybir.AluOpType.add)
                    else:
                        nc.vector.tensor_scalar_add(qT[:, qs], pq[:, :],
                                                    bq[:, 0:1])
                    pk = ps_pv.tile([128, 512], f32, tag="pv")
                    qk_matmuls(pk[:, :], wk, jj)
                    if evac_fold:
                        nc.scalar.activation(kT[:, qs], pk[:, :], Copy)
                    elif fp8_qk:
                        nc.vector.tensor_scalar_mul(kT[:, qs], pk[:, :],
                                                    1.0 / K_SHIFT)
                    else:
                        nc.vector.tensor_copy(kT[:, qs], pk[:, :])

                    for c in range(jj * 4, jj * 4 + 4):       # v over seq chunks
                        v_chunk(c)

                def v_chunk(c):
                    pvreg = ps_pv.tile([128, 512], f32, tag="pv")
                    pv_ = pvreg[:, 0:128]
                    for d in range(DCH):
                        nc.tensor.matmul(pv_, xT[:, d * M + c * 128: d * M + (c + 1) * 128],
                                         wv[:, d * 128:(d + 1) * 128],
                                         start=(d == 0), stop=(d == DCH - 1))
                    # both heads' 64-wide slices in one 3D-AP copy
                    dst = ap3(vv[:, :], c * 65,
                              [[32 * 65, HPC], [1, 64]])
                    src = ap3(pvreg[:, :], 0, [[64, HPC], [1, 64]])
                    nc.vector.tensor_copy(dst, src)

                # ---- attention + output projection ----
                # Score psum groups: chunks x 2 heads interleaved; the two
                # heads' K=64 matmuls are emitted adjacently with different
                # partition bases (0 / 64) so they row-tile into disjoint PE
                # quadrants and different PSUM banks, running concurrently.
                # exp: ACT does 14/16 chunk-units in up-to-2048-wide calls;
                # the last 2 units run on the DVE as a Schraudolph bit-trick
                # exp (round(x*128/ln2 + B) written as int16 = bf16 bits,
                # ~1.8% rms on these logits), freeing ACT, the per-block
                # bottleneck engine.
                if dve_exp == 5:
                    # dve2 split with an s1 tail: the last s0 reader is an
                    # ACT exp, so the next block's first (s0) score matmuls
                    # never wait on this block's DVE exp queue
                    GROUPS = [(2, "s0", "A"), (1, "s1", "A")] * 4 + \
                             [(2, "s0", "A"), (1, "s1", "D"), (1, "s1", "D")]
                elif dve_exp == 4:
                    GROUPS = [(2, "s0", "A"), (1, "s1", "A")] * 4 + \
                             [(2, "s0", "D"), (1, "s1", "D"), (1, "s1", "D")]
                elif dve_exp == 3:
                    # s1-tail: the block ends on s1 groups so the next
                    # block's first (s0) score matmuls don't wait on this
                    # block's last exp.
                    GROUPS = [(2, "s0", "A"), (1, "s1", "A")] * 4 + \
                             [(1, "s0", "A"), (1, "s0", "D"),
                              (1, "s1", "D"), (1, "s1", "D")]
                elif dve_exp == 2:
                    GROUPS = [(2, "s0", "A"), (1, "s1", "A")] * 4 + \
                             [(2, "s0", "A"), (1, "s1", "D"), (1, "s0", "D")]
                else:
                    GROUPS = [(2, "s0", "A"), (1, "s1", "A")] * 5 + \
                             [(1, "s0", "A")]
                EXP_A = 184.66496543257542 * exp_scale  # 2**7/ln(2), descaled
                EXP_B = 16248.75                  # 127*2**7 - c, fitted
                i16 = mybir.dt.int16

                def out_proj(oT, q0):
                    # output projection for these 512 seq rows (4 x 128)
                    for t in range(4):
                        sb = q0 + t * 128
                        oc = stage.tile([128, 1024], out_dt, tag="oc")
                        for half in range(1 if skip_out else 2):
                            po = ps_pv.tile([128, 512], f32, tag="pv")
                            nc.tensor.matmul(po[:, :], oT[:, t * 128:(t + 1) * 128],
                                             wo[:, half * 512:(half + 1) * 512],
                                             start=True, stop=True)
                            # psum->sbuf staging split across DVE and ACT so
                            # neither becomes the attention-phase bottleneck
                            if oc_split and half == 1:
                                nc.scalar.activation(
                                    oc[:, half * 512:(half + 1) * 512],
                                    po[:, :], Copy)
                            else:
                                nc.vector.tensor_copy(
                                    oc[:, half * 512:(half + 1) * 512], po[:, :])
                        nc.gpsimd.dma_start(out_d[sb:sb + 128, :], oc[:, :])

                pending = []

                def attn_block(n, j):
                        q0 = n * S + j * 512
                        # exp lands in two half-block tiles (chunks 0-7 /
                        # 8-15); PV consumes heads interleaved per chunk so
                        # each half releases as soon as both heads pass it.
                        ets = [exp_pool.tile([128, HPC * 8 * 512], bf16,
                                             tag="exp", name=f"et{_i}")
                               for _i in range(2)]
                        c = 0
                        for cnt, pool_name, eng in GROUPS:
                            pool = ps_s0 if pool_name == "s0" else ps_s1
                            ps = pool.tile([128, cnt * HPC * 512], f32, tag=pool_name)
                            for i in range(cnt):
                                k0 = n * S + (c + i) * 128
                                for h in range(HPC):
                                    # serial_scores: TIMING PROBE ONLY (wrong
                                    # math) - both heads from partition base 0
                                    # so no PE row-tiling concurrency possible
                                    hp = slice(0, DH) if serial_scores else \
                                        slice(h * DH, (h + 1) * DH)
                                    nc.tensor.matmul(
                                        ps[:, (i * HPC + h) * 512:(i * HPC + h + 1) * 512],
                                        kT[hp, k0:k0 + 128],
                                        qT[hp, q0:q0 + 512],
                                        start=True, stop=True)
                            et = ets[c // 8]
                            e0 = (c % 8) * HPC * 512
                            ew = cnt * HPC * 512
                            if eng == "A":
                                nc.scalar.activation(
                                    et[:, e0:e0 + ew], ps[:, :], Exp,
                                    scale=exp_scale)
                            else:
                                nc.vector.tensor_scalar(
                                    et[:, e0:e0 + ew].bitcast(i16), ps[:, :],
                                    EXP_A, EXP_B,
                                    mybir.AluOpType.mult, mybir.AluOpType.add)
                            c += cnt
                        # deferred out-proj of the previous block lands here:
                        # its matmuls fill the PE while this block's PV waits
                        # on exp, and the den->recip->bc chain it depended on
                        # has long resolved.
                        while pending:
                            out_proj(*pending.pop(0))
                        oT = oT_pool.tile([128, 512], bf16, tag="oT")
                        det = None if recip_direct else \
                            smalls.tile([1, HPC * 512], f32, tag="den")
                        recip2 = smalls.tile([1, HPC * 512], f32, tag="recip")
                        # PV: heads interleaved per chunk; row 64 = denom
                        pvs = [ps_pv.tile([128, 512], f32, tag="pv",
                                          name=f"pv{_h}")
                               for _h in range(HPC)]
                        for c2 in range(16):
                            et = ets[c2 // 8]
                            e0 = ((c2 % 8) * HPC) * 512
                            for h in range(HPC):
                                vs = (h * 32 + n * 16 + c2) * 65
                                nc.tensor.matmul(
                                    pvs[h][0:65, :],
                                    vv[:, vs:vs + 65],
                                    et[:, e0 + h * 512:e0 + (h + 1) * 512],
                                    start=(c2 == 0), stop=(c2 == 15))
                        for h in range(HPC):
                            if recip_direct:
                                nc.vector.reciprocal_approx_fast(
                                    recip2[:, h * 512:(h + 1) * 512],
                                    pvs[h][64:65, :])
                            else:
                                # stage denominator row to partition 0 (custom
                                # DVE ops drop the input base_partition on HW)
                                nc.vector.tensor_copy(
                                    det[:, h * 512:(h + 1) * 512],
                                    pvs[h][64:65, :])
                        if not recip_direct:
                            nc.vector.reciprocal_approx_fast(recip2[:, :],
                                                             det[:, :])
                        bc2 = smalls.tile([64, HPC * 512], f32, tag="bc")
                        rap = recip2[:, :]
                        nc.sync.dma_start(bc2[:, :], bass.AP(
                            rap.tensor, rap.offset,
                            [[rap.ap[0][0], 1], [0, 64], [1, HPC * 512]]))
                        for h in range(HPC):
                            hp = slice(h * DH, (h + 1) * DH)
                            nc.vector.tensor_mul(oT[hp, :], pvs[h][0:64, :],
                                                 bc2[:, h * 512:(h + 1) * 512])
                        if defer_po:
                            pending.append((oT, q0))
                        else:
                            out_proj(oT, q0)

                # ---- software-pipelined attention (pipe_attn) ----
                # Emission order per period k: 16 single-chunk score groups of
                # block k, with block k-1's 32 PV matmuls and block k-2's 8
                # out-proj matmuls interleaved into the gaps. The PE then
                # never waits on the exp engines: score group g of block k is
                # psum-gated on exp group g-2 of the SAME block (pools s0/s1
                # alternate), and the gaps are filled with PV work whose exp
                # tiles completed last period. The out-proj runs two periods
                # after its PV, so the den->recip->bc2->oT chain (~3us of DVE
                # + DMA latency) is fully hidden. pipe_dve of the 16 exp
                # units go to the DVE Schraudolph path, the rest to ACT.
                dpos = set(int((i + 0.5) * 16 / pipe_dve)
                           for i in range(pipe_dve)) if pipe_dve else set()

                def pipe_pv_mm(m, n_p, ets_p, pvs_p):
                    c2, h = m // 2, m % 2
                    etp = ets_p[c2 // 8]
                    e0p = (c2 % 8) * HPC * 512
                    vs = (h * 32 + n_p * 16 + c2) * 65
                    nc.tensor.matmul(
                        pvs_p[h][0:65, :], vv[:, vs:vs + 65],
                        etp[:, e0p + h * 512:e0p + (h + 1) * 512],
                        start=(c2 == 0), stop=(c2 == 15))

                def pipe_opj_mm(idx, oTp, q0p, st):
                    t, half = idx // 2, idx % 2
                    if half == 0:
                        st["oc"] = stage.tile([128, 1024], out_dt, tag="oc", name="oc")
                    oc = st["oc"]
                    po = ps_pv.tile([128, 512], f32, tag="pv")
                    nc.tensor.matmul(po[:, :], oTp[:, t * 128:(t + 1) * 128],
                                     wo[:, half * 512:(half + 1) * 512],
                                     start=True, stop=True)
                    if oc_split and half == 1:
                        nc.scalar.activation(
                            oc[:, half * 512:(half + 1) * 512], po[:, :], Copy)
                    else:
                        nc.vector.tensor_copy(
                            oc[:, half * 512:(half + 1) * 512], po[:, :])
                    if half == 1:
                        nc.gpsimd.dma_start(out_d[q0p + t * 128:
                                                  q0p + (t + 1) * 128, :],
                                            oc[:, :])

                def pipe_tail(pvs_p):
                    # Denominator rows staged to partitions 0 / 32 (engine
                    # APs need 32-aligned partition bases) by ACT, so the
                    # reciprocal runs on two DVE lanes instead of one.
                    # Lanes 1-31 compute garbage on uninitialized sbuf and
                    # are never read. The chain has a full pipeline period
                    # before the out-proj consumes oT.
                    oT = oT_pool.tile([128, 512], bf16, tag="oT")
                    det = smalls.tile([33, 512], f32, tag="den")
                    recip2 = smalls.tile([33, 512], f32, tag="recip")
                    for h in range(HPC):
                        nc.scalar.activation(det[32 * h:32 * h + 1, :],
                                             pvs_p[h][64:65, :], Copy)
                    nc.vector.reciprocal_approx_fast(recip2[:, :], det[:, :])
                    bc2 = smalls.tile([64, HPC * 512], f32, tag="bc")
                    for h in range(HPC):
                        rap = recip2[32 * h:32 * h + 1, :]
                        nc.sync.dma_start(
                            bc2[:, h * 512:(h + 1) * 512],
                            bass.AP(rap.tensor, rap.offset,
                                    [[rap.ap[0][0], 1], [0, 64], [1, 512]]))
                    for h in range(HPC):
                        hp = slice(h * DH, (h + 1) * DH)
                        nc.vector.tensor_mul(oT[hp, :], pvs_p[h][0:64, :],
                                             bc2[:, h * 512:(h + 1) * 512])
                    return oT

                def attn_pipe(inject=None):
                    blocks = [(n, j) for n in range(NB) for j in range(4)]
                    prev = None      # (n, ets, q0) scored last iteration
                    prev_oT = None   # (oT, q0) awaiting out-proj
                    for bi, (n, j) in enumerate(blocks):
                        q0 = n * S + j * 512
                        ets = [exp_pool.tile([128, HPC * 8 * 512], bf16,
                                             tag="exp", name=f"et{_i}")
                               for _i in range(2)]
                        pvs_p = None
                        if prev is not None:
                            pvs_p = [ps_pv.tile([128, 512], f32, tag="pv",
                                                name=f"pv{_h}")
                                     for _h in range(HPC)]
                        opj_st = {}
                        for g in range(16):
                            pool = ps_s0 if g % 2 == 0 else ps_s1
                            ps = pool.tile([128, HPC * 512], f32,
                                           tag="s0" if g % 2 == 0 else "s1")
                            k0 = n * S + g * 128
                            for h in range(HPC):
                                hp = slice(0, DH) if serial_scores else \
                                    slice(h * DH, (h + 1) * DH)
                                nc.tensor.matmul(
                                    ps[:, h * 512:(h + 1) * 512],
                                    kT[hp, k0:k0 + 128], qT[hp, q0:q0 + 512],
                                    start=True, stop=True)
                            et = ets[g // 8]
                            e0 = (g % 8) * HPC * 512
                            ew = HPC * 512
                            if g in dpos:
                                nc.vector.tensor_scalar(
                                    et[:, e0:e0 + ew].bitcast(i16), ps[:, :],
                                    EXP_A, EXP_B,
                                    mybir.AluOpType.mult, mybir.AluOpType.add)
                            else:
                                nc.scalar.activation(
                                    et[:, e0:e0 + ew], ps[:, :], Exp,
                                    scale=exp_scale)
                            if prev is not None:
                                n_p, ets_p, q0_p = prev
                                if g < 8:   # 3 PV matmuls per early gap
                                    for m in range(3 * g, 3 * g + 3):
                                        pipe_pv_mm(m, n_p, ets_p, pvs_p)
                                else:       # 1 PV + 1 out-proj per late gap
                                    pipe_pv_mm(24 + g - 8, n_p, ets_p, pvs_p)
                                    if prev_oT is not None:
                                        pipe_opj_mm(g - 8, prev_oT[0],
                                                    prev_oT[1], opj_st)
                        if inject and bi in inject:
                            inject[bi]()
                        if prev is not None:
                            n_p, ets_p, q0_p = prev
                            prev_oT = (pipe_tail(pvs_p), q0_p)
                        prev = (n, ets, q0)
                    # drain: PV + tail of the last block, final out-projs.
                    # The last block's PV stalls chunk-by-chunk on its own
                    # exp (there is no next block's scores to pace it), so
                    # the previous block's out-proj matmuls are interleaved
                    # into those stalls.
                    n_p, ets_p, q0_p = prev
                    pvs_p = [ps_pv.tile([128, 512], f32, tag="pv",
                                        name=f"pv{_h}")
                             for _h in range(HPC)]
                    opj_st = {}
                    for m in range(32):
                        pipe_pv_mm(m, n_p, ets_p, pvs_p)
                        if prev_oT is not None and m % 4 == 3:
                            pipe_opj_mm(m // 4, prev_oT[0], prev_oT[1],
                                        opj_st)
                    out_proj(pipe_tail(pvs_p), q0_p)

                if pipe_attn and pipe_ilv:
                    # batch-1 projections stream in during the first four
                    # attention blocks (batch 0), overlapping the input DMA
                    # and projection matmuls with the attention pipeline
                    for jj in range(4):
                        proj_block(jj)
                    attn_pipe(inject={
                        bi: (lambda jj=jj: proj_block(jj))
                        for bi, jj in enumerate(range(4, 8))})
                elif pipe_attn:
                    for jj in range(8):
                        proj_block(jj)
                    attn_pipe()
                elif interleave:
                    for jj in range(4):
                        proj_block(jj)
                    attn_block(0, 0)
                    attn_block(0, 1)
                    proj_block(4)
                    proj_block(5)
                    attn_block(0, 2)
                    proj_block(6)
                    proj_block(7)
                    attn_block(0, 3)
                    for j in range(4):
                        attn_block(1, j)
                else:
                    for jj in range(8):
                        proj_block(jj)
                    for n in range(NB):
                        for j in range(4):
                            attn_block(n, j)
                while pending:
                    out_proj(*pending.pop(0))

    nc.compile()
    return nc


def _get_compiled():
    global _compiled
    if _compiled is None:
        _compiled = _build()
    return _compiled


def _pack_rows(a):
    """[D, N] -> [128, DCH*N] chunk-major: out[p, d*N+c] = a[d*128+p, c]."""
    Drows, N = a.shape
    ch = Drows // 128
    return np.ascontiguousarray(
        a.reshape(ch, 128, N).transpose(1, 0, 2).reshape(128, ch * N))


def _prep_in_maps(x, wq, bq, wk, wv, wo, fp8_qk=None, x8_onchip=None,
                  act_evac=None):
    import inspect
    if fp8_qk is None:
        fp8_qk = inspect.signature(_build).parameters["fp8_qk"].default
    if x8_onchip is None:
        x8_onchip = inspect.signature(_build).parameters["x8_onchip"].default
    if act_evac is None:
        act_evac = inspect.signature(_build).parameters["act_evac"].default
    # with act_evac the kernel keeps qT at the 2^13 weight prescale, so bq
    # must carry the same prescale (exp descales both together)
    bq_scale = SCALE * (QK_SHIFT if (act_evac and fp8_qk) else 1.0)
    F8 = ml_dtypes.float8_e4m3
    xTf = x.reshape(M, D).T
    xT = _pack_rows(xTf.astype(BF16))
    x8 = _pack_rows(np.clip(xTf, -240, 240).astype(F8)) \
        if (fp8_qk and not x8_onchip) else None
    maps = []
    for i in range(N_CORES):
        rs = slice(i * FS, (i + 1) * FS)
        if fp8_qk:
            wqT = _pack_rows((wq[rs, :] * (SCALE * QK_SHIFT)).T.astype(F8))
            wkT = _pack_rows((wk[rs, :] * K_SHIFT).T.astype(F8))
        else:
            wqT = _pack_rows((wq[rs, :] * SCALE).T.astype(BF16))
            wkT = _pack_rows(wk[rs, :].T.astype(BF16))
        m = {
            "xT": xT,
            "wqT": wqT,
            "wkT": wkT,
            "wvT": _pack_rows(wv[rs, :].T.astype(BF16)),
            "woT": np.ascontiguousarray(wo[:, rs].T).astype(BF16),
            "bq": (bq[rs] * bq_scale).astype(np.float32).reshape(FS, 1),
            "vtag": np.zeros((1, _VTAG), np.float32),
        }
        if fp8_qk and x8 is not None:
            m["x8"] = x8
        maps.append(m)
    return maps


def kernel(x, wq, bq, wk, bk, wv, bv, wo, bo, _want_results=False, _trace=False):
    from concourse.bass_utils import run_bass_kernel_spmd

    x = np.asarray(x, dtype=np.float32)
    wq = np.asarray(wq, dtype=np.float32)
    bq = np.asarray(bq, dtype=np.float32)
    wk = np.asarray(wk, dtype=np.float32)
    wv = np.asarray(wv, dtype=np.float32)
    wo = np.asarray(wo, dtype=np.float32)
    bv = np.asarray(bv, dtype=np.float32)
    bo = np.asarray(bo, dtype=np.float32)

    nc = _get_compiled()
    import inspect
    _fp8_default = inspect.signature(_build).parameters["fp8_qk"].default
    in_maps = _prep_in_maps(x, wq, bq, wk, wv, wo, fp8_qk=_fp8_default)
    res = None
    for attempt in range(3):
        try:
            res = run_bass_kernel_spmd(nc, in_maps, list(range(N_CORES)),
                                       trace=_trace)
            break
        except Exception:
            # the shared device occasionally reports
            # NRT_EXEC_UNIT_UNRECOVERABLE transiently; back off and retry
            if attempt == 2:
                raise
            import time as _time
            _time.sleep(15)

    acc = np.zeros((M, D), dtype=np.float32)
    for i in range(N_CORES):
        acc += res.results[i]["out"].astype(np.float32)
    acc += bo + bv @ wo.T
    out = acc.reshape(NB, S, D)
    if _want_results:
        return out, res
    return out

